# revision 8
# baseline (speedup 1.0000x reference)
"""Causal single-head attention (B=4, T=4096, D_in=1024, D_out=64) on 8 trn2 cores.

Sharding: 2 cores per batch. Within a pair, core h in {0,1} owns the k/v
positions in 256-wide blocks of parity h (even/odd), and computes partial
unnormalized attention for ALL 4096 queries over its k half, plus the
softmax row-sums (via a ones-column appended to V). The host sums the two
partials and normalizes. Causality lands symmetrically on both parities, so
one SPMD program (identical instruction stream) serves all 8 cores; per-core
behavior differs only through data:

  - xT (x[b] transposed to [D,T]) with each 512-column tile's two 256-blocks
    swapped for h=1, so "even permuted block" = own-parity block on every core
  - iota_q (global query index per Q^T column) and kg (global key index per
    K^T row) tables driving the causal mask compare

All matmuls run in float32r (full PE rate for free-dim>=256, ~1.3e-4 rel err).
Projection stripes (1024 t-columns) interleave with attention q-slots so PE
stays warm and DMA overlaps compute. Per q-slot qt (512 queries, 2*qt+2
k-tiles of 128):
  scores^T tile = K^T-slice(64x128) x Q^T-slot(64x512) -> PSUM
  exp(0.125*s) on ACT (PSUM -> SBUF, groups of 2 tiles)
  causal mask only on the last 2 k-tiles (the only ones crossing the
  diagonal -- uniform across cores): (iota_q >= kg) * exp on DVE
  out^T(65x512) += V_aug(128x65) x exp^T(128x512), accumulated over k-tiles,
  attnV lagging one group behind scores so PE never waits on ACT.
"""

import numpy as np

B, T, D, E = 4, 4096, 1024, 64
NCORES = 8
P = 128
HB = 256  # parity half-block width
NQT = 8  # q-slots of 512
DC = D // P  # 8 d-chunks
GRP = 2  # k-tiles per exp group

_cache = {}


def _sl(start, size):
    return slice(start, start + size)


def _build_program():
    import concourse.mybir as mybir
    import concourse.tile as tile
    from concourse import bacc

    f32 = mybir.dt.float32
    f32r = mybir.dt.float32r
    Exp = mybir.ActivationFunctionType.Exp
    Alu = mybir.AluOpType

    nc = bacc.Bacc("TRN2", target_bir_lowering=False, debug=False, num_devices=NCORES)

    xT = nc.dram_tensor("xT", [D, T], f32r, kind="ExternalInput")
    wkv = nc.dram_tensor("wkv", [DC, P, 2 * E], f32r, kind="ExternalInput")
    wq = nc.dram_tensor("wq", [DC, P, E], f32r, kind="ExternalInput")
    kg = nc.dram_tensor("kg", [P, 16], f32, kind="ExternalInput")
    iota_q = nc.dram_tensor("iota_q", [P, T], f32, kind="ExternalInput")
    ident = nc.dram_tensor("ident", [P, 64], f32r, kind="ExternalInput")
    ones = nc.dram_tensor("ones", [P, 16], f32r, kind="ExternalInput")
    out = nc.dram_tensor("out", [E + 1, T], f32, kind="ExternalOutput")

    with tile.TileContext(nc) as tc:
        with (
            tc.tile_pool(name="const", bufs=1) as cpool,
            tc.tile_pool(name="persist", bufs=1) as ppool,
            tc.tile_pool(name="xt", bufs=24) as xtpool,
            tc.tile_pool(name="kvps", bufs=1, space="PSUM") as kvps,
            tc.tile_pool(name="qps", bufs=1, space="PSUM") as qps,
            tc.tile_pool(name="sps", bufs=2, space="PSUM") as sps,
            tc.tile_pool(name="ops", bufs=1, space="PSUM") as ops,
            tc.tile_pool(name="exp", bufs=3) as exppool,
        ):
            wkv_sb = cpool.tile([P, DC, 2 * E], f32r)
            nc.sync.dma_start(wkv_sb[:], wkv.ap().rearrange("c p w -> p c w"))
            wq_sb = cpool.tile([P, DC, E], f32r)
            nc.sync.dma_start(wq_sb[:], wq.ap().rearrange("c p w -> p c w"))
            kg_sb = cpool.tile([P, 16], f32)
            nc.sync.dma_start(kg_sb[:], kg.ap())
            iq_sb = cpool.tile([P, T], f32)
            nc.sync.dma_start(iq_sb[:], iota_q.ap())
            ident_sb = cpool.tile([P, 64], f32r)
            nc.sync.dma_start(ident_sb[:], ident.ap())

            kT_sb = ppool.tile([E, T // 2], f32r, name="kT")
            qT_sb = ppool.tile([E, T], f32r, name="qT")
            vT_tmp = ppool.tile([P, T // 2], f32r, name="vTt")  # rows 64..127 used
            V_sb = ppool.tile([P, 16, E + 1], f32r, name="V")
            out_sb = ppool.tile([E + 1, T], f32, name="outsb")

            # softmax-denominator ones column
            nc.sync.dma_start(V_sb[:, :, E], ones.ap())

            xT_view = xT.ap().rearrange("(c p) t -> c p t", p=P)
            stripes = [None] * 4  # per-stripe xt tiles

            def issue_stripe_dma(t2):
                xts = []
                for dc in range(DC):
                    xt_t = xtpool.tile([P, 1024], f32r)
                    nc.sync.dma_start(
                        xt_t[:], xT_view[dc, :, 1024 * t2 : 1024 * (t2 + 1)]
                    )
                    xts.append(xt_t)
                stripes[t2] = xts

            def issue_stripe_proj(t2):
                xts = stripes[t2]
                for half in range(2):
                    # K^T|V^T over the own-parity 256-block (even position)
                    kv = kvps.tile([P, HB], f32)
                    for dc in range(DC):
                        nc.tensor.matmul(
                            kv[:],
                            wkv_sb[:, dc, :],
                            xts[dc][:, _sl(512 * half, HB)],
                            start=(dc == 0),
                            stop=(dc == DC - 1),
                        )
                    m = 2 * t2 + half
                    nc.vector.tensor_copy(kT_sb[:, _sl(HB * m, HB)], kv[0:E, :])
                    nc.vector.tensor_copy(vT_tmp[E:P, _sl(HB * m, HB)], kv[E:P, :])
                    # Q^T over the full 512-tile
                    q = qps.tile([E, 512], f32)
                    for dc in range(DC):
                        nc.tensor.matmul(
                            q[:],
                            wq_sb[:, dc, :],
                            xts[dc][:, _sl(512 * half, 512)],
                            start=(dc == 0),
                            stop=(dc == DC - 1),
                        )
                    qt_i = 2 * t2 + half
                    nc.vector.tensor_copy(qT_sb[:, _sl(512 * qt_i, 512)], q[:])
                # V^T -> V via PE transpose (4 x 128-col pieces)
                for j in range(4 * t2, 4 * t2 + 4):
                    vt = kvps.tile([P, E], f32r, tag="vt")
                    nc.tensor.transpose(
                        vt[:], vT_tmp[E:P, _sl(P * j, P)], ident_sb[E:P, :]
                    )
                    nc.vector.tensor_copy(V_sb[:, j, 0:E], vt[:])

            pending = None  # (qt, j0, g, nkb, exp_tile, po)

            def issue_attnv(pend):
                qt, j0, g, nkb, ex, po_t = pend
                for jj in range(g):
                    j2 = j0 + jj
                    nc.tensor.matmul(
                        po_t[:],
                        V_sb[:, j2, :],
                        ex[:, _sl(512 * jj, 512)],
                        start=(j2 == 0),
                        stop=(j2 == nkb - 1),
                    )
                if j0 + g == nkb:
                    nc.vector.tensor_copy(out_sb[:, _sl(512 * qt, 512)], po_t[:])

            def issue_slot(qt):
                nonlocal pending
                nkb = 2 * qt + 2
                po = ops.tile([E + 1, 512], f32)
                qsl = qT_sb[:, _sl(512 * qt, 512)]
                for j0 in range(0, nkb, GRP):
                    g = min(GRP, nkb - j0)
                    ps = sps.tile([P, 512 * GRP], f32)
                    for jj in range(g):
                        j2 = j0 + jj
                        nc.tensor.matmul(
                            ps[:, _sl(512 * jj, 512)],
                            kT_sb[:, _sl(P * j2, P)],
                            qsl,
                            start=True,
                            stop=True,
                        )
                    ex = exppool.tile([P, 512 * GRP], f32r)
                    nc.scalar.activation(
                        ex[:, : 512 * g], ps[:, : 512 * g], Exp, scale=0.125
                    )
                    if j0 + g == nkb:
                        # only the last two k-tiles cross the diagonal
                        for jj in range(g):
                            j2 = j0 + jj
                            nc.vector.scalar_tensor_tensor(
                                out=ex[:, _sl(512 * jj, 512)],
                                in0=iq_sb[:, _sl(512 * qt, 512)],
                                scalar=kg_sb[:, j2 : j2 + 1],
                                in1=ex[:, _sl(512 * jj, 512)],
                                op0=Alu.is_ge,
                                op1=Alu.mult,
                            )
                    if pending is not None:
                        issue_attnv(pending)
                    pending = (qt, j0, g, nkb, ex, po)

            # --- schedule: deep DMA prefetch, stripes interleaved with slots
            issue_stripe_dma(0)
            issue_stripe_dma(1)
            issue_stripe_dma(2)
            issue_stripe_proj(0)
            for seg in range(4):
                if seg == 0:
                    issue_stripe_dma(3)
                issue_slot(2 * seg)
                issue_slot(2 * seg + 1)
                if seg < 3:
                    issue_stripe_proj(seg + 1)
            issue_attnv(pending)

            nc.sync.dma_start(out.ap(), out_sb[:])

    nc.compile()
    return nc


def _host_inputs():
    """Core-independent pieces + per-parity mask tables (iota_q, kg)."""
    ident = np.zeros((P, 64), dtype=np.float32)
    for p in range(P):
        ident[p, p % 64] = 1.0
    iqs, kgs = [], []
    ii = np.arange(P, dtype=np.float32)
    for h in range(2):
        # global query index of each (permuted) Q^T column, bcast over rows
        jl = np.arange(T)
        r = (jl // HB) % 2
        gq = 2 * (jl // 512) + (r ^ h)
        qglob = (HB * gq + jl % HB).astype(np.float32)
        iqs.append(np.broadcast_to(qglob, (P, T)).copy())
        # global key index of each K^T row, per 128-wide k-tile j2
        kg = np.zeros((P, 16), dtype=np.float32)
        for j2 in range(16):
            g_k = 2 * (j2 // 2) + h
            kg[:, j2] = HB * g_k + P * (j2 % 2) + ii
        kgs.append(kg)
    return ident, iqs, kgs


def kernel(x, Wq, Wk, Wv):
    from concourse.bass_utils import run_bass_kernel_spmd

    if "nc" not in _cache:
        _cache["nc"] = _build_program()
    nc = _cache["nc"]

    x = np.asarray(x, dtype=np.float32)
    Wq = np.asarray(Wq, dtype=np.float32)
    Wk = np.asarray(Wk, dtype=np.float32)
    Wv = np.asarray(Wv, dtype=np.float32)

    wkv = np.ascontiguousarray(np.concatenate([Wk, Wv], axis=1).reshape(DC, P, 2 * E))
    wq = np.ascontiguousarray(Wq.reshape(DC, P, E))
    ident, iqs, kgs = _host_inputs()
    ones = np.ones((P, 16), dtype=np.float32)

    xT_all = x.transpose(0, 2, 1)  # [B, D, T]
    in_maps = []
    for c in range(NCORES):
        b, h = c // 2, c % 2
        xT = xT_all[b]
        if h == 1:  # swap 256-pairs so own-parity block is at even positions
            xT = xT.reshape(D, 8, 2, HB)[:, :, ::-1, :].reshape(D, T)
        in_maps.append(
            {
                "xT": np.ascontiguousarray(xT),
                "wkv": wkv,
                "wq": wq,
                "kg": kgs[h],
                "iota_q": iqs[h],
                "ident": ident,
                "ones": ones,
            }
        )

    res = run_bass_kernel_spmd(nc, in_maps, list(range(NCORES)))
    _cache["last_res"] = res

    outp = np.empty((B, T, E), dtype=np.float32)
    for b in range(B):
        U = np.zeros((E + 1, T), dtype=np.float64)
        for h in range(2):
            u = res.results[2 * b + h]["out"].astype(np.float64)
            if h == 1:
                u = u.reshape(E + 1, 8, 2, HB)[:, :, ::-1, :].reshape(E + 1, T)
            U += u
        outp[b] = (U[:E] / U[E : E + 1]).T.astype(np.float32)
    return outp
